# revision 32
# baseline (speedup 1.0000x reference)
"""Causal attention (B=4, N=2048, D=1024) on 8 Trainium2 NeuronCores.

v2 design (vs baseline):
  * All on-chip tensors bf16 (tolerance 2e-2; measured numpy pipeline err
    ~4e-3).  Halves DMA bytes and SBUF so K^T, V and Q^T stay fully
    SBUF-resident -- no DRAM spill roundtrips.
  * Scores computed TRANSPOSED (S^T[k,q] per key tile): the exp'd P^T is
    directly the stationary operand of the AV matmul, eliminating all PE
    transposes and the exp->transpose->copy->AV serial chain.  Row sums
    come from a 1-wide matmul against a ones vector that reuses the AV
    stationary (P^T) already loaded in the array.
  * Core 2b+s handles batch b; s=0 takes query tiles {0,2,4,6, 9,11,13,15},
    s=1 takes {1,3,5,7, 8,10,12,14} -- both sum to 68 causal key-tile pairs.
    The program is SPMD-uniform with key limits (2,4,..,16); the per-core
    diagonal/full masks are passed as input data ([128,512] = group1 pair +
    group2 pair of 128-col blocks).
  * Slot groups of 4 give 512-wide moving operands for S^T; widths taper
    (512/384/256/128) following the causal staircase.
  * x double-buffered across the two key-tile phases; weights loaded once.
"""
import sys

sys.path.insert(0, "/opt/trn_rl_repo")

from contextlib import ExitStack

import numpy as np
import ml_dtypes

import concourse.bass as bass
import concourse.mybir as mybir
import concourse.tile as tile
from concourse import bacc
from concourse.bass_utils import run_bass_kernel_spmd

B, N, D = 4, 2048, 1024
N_CORES = 8
N_SLOTS = 8
N_KTILES = 16
SCALE = 1.0 / 32.0   # 1/sqrt(D)
NEG = -1.0e9

F32 = mybir.dt.float32
BF16 = mybir.dt.bfloat16
BF = ml_dtypes.bfloat16

# query-tile sets per parity slot s (ascending); both have sum(g+1) == 68
QSETS = [
    [0, 2, 4, 6, 9, 11, 13, 15],
    [1, 3, 5, 7, 8, 10, 12, 14],
]
# uniform program limits per slot (key tiles 0..L-1 computed)
LIMITS = [2, 4, 6, 8, 10, 12, 14, 16]

_NC_CACHE = {}
TRACE = False
LAST_EXEC_NS = None


def _build_nc():
    nc = bacc.Bacc(None, target_bir_lowering=False, debug=False)

    # x tile layout: [tile, p=d%128, dchunk, token]
    x_t = nc.declare_dram_parameter("x_t", [N_KTILES, 128, 8, 128], BF16, isOutput=False)
    x_qt = nc.declare_dram_parameter("x_qt", [N_SLOTS, 128, 8, 128], BF16, isOutput=False)
    # weights: [p=d%128, dchunk, ecol]
    wq = nc.declare_dram_parameter("wq", [128, 8, 1024], BF16, isOutput=False)
    wk = nc.declare_dram_parameter("wk", [128, 8, 1024], BF16, isOutput=False)
    # wv is e-half-major so each half is one contiguous DMA on its own queue
    wv = nc.declare_dram_parameter("wv", [2, 128, 8, 512], BF16, isOutput=False)
    mask_in = nc.declare_dram_parameter("mask", [128, 512], F32, isOutput=False)
    out_q = nc.declare_dram_parameter("out_q", [N_SLOTS, 128, D], BF16, isOutput=True)

    with tile.TileContext(nc) as tc, ExitStack() as top:
        consts = top.enter_context(tc.tile_pool(name="consts", bufs=1))
        kt_pool = top.enter_context(tc.tile_pool(name="ktp", bufs=1))
        v_pool = top.enter_context(tc.tile_pool(name="vp", bufs=1))
        qt_pool = top.enter_context(tc.tile_pool(name="qtp", bufs=1))

        ones = consts.tile([128, 8], BF16)
        nc.vector.memset(ones, 1.0)
        mask_sb = consts.tile([128, 512], F32)
        dummy = consts.tile([128, 512], BF16)
        nc.vector.memset(dummy, 0.0)

        KT = kt_pool.tile([128, 8, N], BF16)     # [p=e%128, echunk, key]
        V = v_pool.tile([128, N_KTILES, D], BF16)  # [p=key%128, ktile, e]
        QT = qt_pool.tile([128, 8, 1024], BF16)  # [p=e%128, echunk, qcol]

        with ExitStack() as ph12:
            xt_pool = ph12.enter_context(tc.tile_pool(name="xtp", bufs=2))
            qxt_pool = ph12.enter_context(tc.tile_pool(name="qxt", bufs=1))
            w_pool = ph12.enter_context(tc.tile_pool(name="wp", bufs=1))
            ps_mm = ph12.enter_context(tc.tile_pool(name="ps_mm", bufs=8, space="PSUM"))

            # spread weight DMAs across queues so they stream in parallel
            # (per-queue DMA BW is ~100-180 GB/s, well under core BW).
            # wv gates the kernel's first PE work: contiguous 1MB halves on
            # two queues land ~5us earlier than one 2MB transfer.
            # first wv half split across two queues (it gates the first PE
            # chain); second half + the rest stream behind
            wv_sb = w_pool.tile([128, 2, 8, 512], BF16, tag="wv")
            nc.scalar.dma_start(out=wv_sb[:, 0, 0:4, :], in_=wv[0][:, 0:4, :])
            nc.gpsimd.dma_start(out=wv_sb[:, 0, 4:8, :], in_=wv[0][:, 4:8, :])
            nc.sync.dma_start(out=wv_sb[:, 1], in_=wv[1][:, :, :])
            wk_sb = w_pool.tile([128, 8, 1024], BF16, tag="wk")
            wq_sb = w_pool.tile([128, 8, 1024], BF16, tag="wq")
            nc.sync.dma_start(out=wq_sb, in_=wq[:, :, :])

            # HAM prewarm: keep the PE busy on throwaway matmuls while the
            # first weight/x DMAs land, so real chains start at 2.4 GHz
            ps_warm = ps_mm.tile([128, 512], F32, tag="mm", name="warm")
            for i in range(48):
                nc.tensor.matmul(
                    ps_warm, dummy[:, 0:128], dummy,
                    start=(i == 0), stop=(i == 47),
                )

            QXT = qxt_pool.tile([128, 8, 8, 128], BF16, tag="qx")
            nc.sync.dma_start(
                out=QXT, in_=x_qt[:].rearrange("s p c q -> p s c q")
            )
            nc.sync.dma_start(out=mask_sb, in_=mask_in[:, :])

            def proj_phase(kh):
                xT = xt_pool.tile([128, 8, 8, 128], BF16, tag="xT", name=f"x{kh}")
                if kh == 0:
                    # per-tile DMAs: first V chain starts after one tile lands
                    for lt in range(8):
                        nc.gpsimd.dma_start(
                            out=xT[:, lt, :, :], in_=x_t[lt][:, :, :]
                        )
                    # wk rides the gpsimd queue behind the kh0 x tiles;
                    # K proj needs it only after the 8 V chains (~25us in)
                    nc.gpsimd.dma_start(out=wk_sb, in_=wk[:, :, :])
                else:
                    # bulk prefetch on the scalar queue (free after wv)
                    nc.scalar.dma_start(
                        out=xT, in_=x_t[8:16].rearrange("t p c q -> p t c q")
                    )
                # V projection: stationary x chunk shared across both e-halves.
                # The first two kh0 tiles run e-halves sequentially so the
                # first chain depends only on the first wv half-DMA.
                for lt in range(8):
                    t = kh * 8 + lt
                    vps = [ps_mm.tile([128, 512], F32, tag="mm", name=f"v{t}_{eh}")
                           for eh in range(2)]
                    if kh == 0 and lt < 2:
                        for eh in range(2):
                            for c in range(8):
                                nc.tensor.matmul(
                                    vps[eh], xT[:, lt, c, :], wv_sb[:, eh, c, :],
                                    start=(c == 0), stop=(c == 7),
                                )
                    else:
                        for c in range(8):
                            for eh in range(2):
                                nc.tensor.matmul(
                                    vps[eh], xT[:, lt, c, :], wv_sb[:, eh, c, :],
                                    start=(c == 0), stop=(c == 7),
                                )
                    for eh in range(2):
                        nc.vector.tensor_copy(V[:, t, eh * 512:(eh + 1) * 512], vps[eh])
                # K^T projection: stationary W chunk shared across both key groups
                for e in range(8):
                    kps = [ps_mm.tile([128, 512], F32, tag="mm", name=f"k{kh}_{e}_{g}")
                           for g in range(2)]
                    for c in range(8):
                        for kg in range(2):
                            nc.tensor.matmul(
                                kps[kg], wk_sb[:, c, e * 128:(e + 1) * 128],
                                xT[:, kg * 4:(kg + 1) * 4, c, :],
                                start=(c == 0), stop=(c == 7),
                            )
                    for kg in range(2):
                        key0 = (kh * 8 + kg * 4) * 128
                        nc.vector.tensor_copy(KT[:, e, key0:key0 + 512], kps[kg])

            def proj_queries():
                # both slot groups; stationary W chunk shared across groups
                for e in range(8):
                    qps = [ps_mm.tile([128, 512], F32, tag="mm", name=f"q{e}_{g}")
                           for g in range(2)]
                    for c in range(8):
                        for g in range(2):
                            nc.tensor.matmul(
                                qps[g], wq_sb[:, c, e * 128:(e + 1) * 128],
                                QXT[:, g * 4:(g + 1) * 4, c, :],
                                start=(c == 0), stop=(c == 7),
                            )
                    for g in range(2):
                        nc.vector.tensor_copy(QT[:, e, g * 512:(g + 1) * 512], qps[g])

            proj_phase(0)
            proj_queries()
            proj_phase(1)

        # ---- attention: S^T per key tile, then AV with P^T stationary ----
        with ExitStack() as ph3:
            pt_pool = ph3.enter_context(tc.tile_pool(name="ptp", bufs=1))
            ps_st = ph3.enter_context(tc.tile_pool(name="ps_st", bufs=3, space="PSUM"))
            ps_o = ph3.enter_context(tc.tile_pool(name="ps_o", bufs=2, space="PSUM"))
            ps_rs = ph3.enter_context(tc.tile_pool(name="ps_rs", bufs=1, space="PSUM"))
            sc_pool = ph3.enter_context(tc.tile_pool(name="scp", bufs=2))
            outp = ph3.enter_context(tc.tile_pool(name="outp", bufs=2))

            PTs = [
                pt_pool.tile([128, 8, 512], BF16, tag="pt1", name="PT1"),
                pt_pool.tile([128, 16, 512], BF16, tag="pt2", name="PT2"),
            ]

            def st_fused():
                # one pass over key tiles; each KT stationary chunk serves
                # BOTH slot groups' S^T matmuls (halves LDWEIGHTS for kt<8)
                for kt in range(16):
                    work = []   # (group, sps, w, col0, f)
                    for g in ((1, 0) if kt < 8 else (1,)):
                        Ls = LIMITS[g * 4:(g + 1) * 4]
                        f = sum(1 for L in Ls if L <= kt)
                        w = (4 - f) * 128
                        col0 = f * 128
                        sps = ps_st.tile([128, 512], F32, tag="st",
                                         name=f"s{g}_{kt}")
                        work.append((g, sps, w, col0, f))
                    for c in range(8):
                        for g, sps, w, col0, f in work:
                            nc.tensor.matmul(
                                sps[:, 0:w],
                                KT[:, c, kt * 128:(kt + 1) * 128],
                                QT[:, c, g * 512 + col0: g * 512 + col0 + w],
                                start=(c == 0), stop=(c == 7),
                            )
                    for g, sps, w, col0, f in work:
                        Ls = LIMITS[g * 4:(g + 1) * 4]
                        if kt == Ls[f] - 2:
                            nc.vector.tensor_add(
                                sps[:, 0:128], sps[:, 0:128],
                                mask_sb[:, g * 256: g * 256 + 128],
                            )
                        elif kt == Ls[f] - 1:
                            nc.vector.tensor_add(
                                sps[:, 0:128], sps[:, 0:128],
                                mask_sb[:, g * 256 + 128: g * 256 + 256],
                            )
                        nc.scalar.activation(
                            PTs[g][:, kt, col0:col0 + w], sps[:, 0:w],
                            mybir.ActivationFunctionType.Exp,
                            bias=0.0, scale=SCALE,
                        )

            def av_slot(g, j):
                PT = PTs[g]
                slot = g * 4 + j
                L = LIMITS[slot]
                col = j * 128
                O_ps = ps_o.tile([128, D], F32, tag="O", name=f"O{slot}")
                rs_ps = ps_rs.tile([128, 1], F32, tag="rs", name=f"r{slot}")
                for kt in range(L):
                    pt_blk = PT[:, kt, col:col + 128]
                    for h in range(2):
                        nc.tensor.matmul(
                            O_ps[:, h * 512:(h + 1) * 512], pt_blk,
                            V[:, kt, h * 512:(h + 1) * 512],
                            start=(kt == 0), stop=(kt == L - 1),
                        )
                    nc.tensor.matmul(
                        rs_ps, pt_blk, ones[:, 0:1],
                        start=(kt == 0), stop=(kt == L - 1),
                    )
                stats = sc_pool.tile([128, 8], F32, tag="stats", name=f"st{slot}")
                recip = stats[:, 0:1]
                nc.vector.reciprocal(recip, rs_ps)
                out_sb = outp.tile([128, D], BF16, tag="osb", name=f"ou{slot}")
                nc.vector.tensor_scalar_mul(out_sb, O_ps, recip)
                # alternate output queues so the final drain is parallel
                eng = nc.sync if slot % 2 == 0 else nc.gpsimd
                eng.dma_start(out=out_q[slot][:, :], in_=out_sb)

            # descending L within each group: the big slots' outputs DMA out
            # early, shrinking the end-of-kernel drain
            # group2 first and descending L within groups: outputs of the big
            # slots DMA out early; the last emitted slot (L=2) has the
            # shortest end-of-kernel chain
            st_fused()
            for j in (3, 2, 1, 0):
                av_slot(1, j)
            for j in (3, 2, 1, 0):
                av_slot(0, j)

    nc.compile()
    return nc


def _masks():
    k = np.arange(128)[:, None]
    q = np.arange(128)[None, :]
    tril_t = np.where(k <= q, 0.0, NEG).astype(np.float32)  # S^T diag block
    fullneg = np.full((128, 128), NEG, np.float32)
    zeros = np.zeros((128, 128), np.float32)
    m_s0 = np.concatenate([tril_t, fullneg, zeros, tril_t], axis=1)
    m_s1 = np.concatenate([zeros, tril_t, tril_t, fullneg], axis=1)
    return m_s0, m_s1


def kernel(x, Wq, Wk, Wv):
    global LAST_EXEC_NS
    x = np.asarray(x, dtype=np.float32)
    Wq = np.asarray(Wq, dtype=np.float32)
    Wk = np.asarray(Wk, dtype=np.float32)
    Wv = np.asarray(Wv, dtype=np.float32)

    if "nc" not in _NC_CACHE:
        _NC_CACHE["nc"] = _build_nc()
    nc = _NC_CACHE["nc"]

    # host pre-transpose: x[b] (N, D) -> (tile, p=d%128, dchunk, token), bf16
    xt_all = np.ascontiguousarray(
        x.reshape(B, N_KTILES, 128, 8, 128).transpose(0, 1, 4, 3, 2).astype(BF)
    )  # [B, tile, p, c, q]

    # weights -> [p=d%128, dchunk, ecol], bf16
    wq_r = np.ascontiguousarray(Wq.reshape(8, 128, 1024).transpose(1, 0, 2).astype(BF))
    wk_r = np.ascontiguousarray(Wk.reshape(8, 128, 1024).transpose(1, 0, 2).astype(BF))
    # wv: [eh, p=d%128, dchunk, ecol]
    wv_r = np.ascontiguousarray(
        Wv.reshape(8, 128, 2, 512).transpose(2, 1, 0, 3).astype(BF))

    m_s0, m_s1 = _masks()
    in_maps = []
    for c in range(N_CORES):
        b, s = divmod(c, 2)
        in_maps.append({
            "x_t": xt_all[b],
            "x_qt": np.ascontiguousarray(xt_all[b, QSETS[s]]),
            "wq": wq_r, "wk": wk_r, "wv": wv_r,
            "mask": m_s1 if s else m_s0,
        })

    res = run_bass_kernel_spmd(nc, in_maps, list(range(N_CORES)), trace=TRACE)
    LAST_EXEC_NS = res.exec_time_ns

    out = np.empty((B, N, D), dtype=np.float32)
    for c in range(N_CORES):
        b, s = divmod(c, 2)
        oq = np.asarray(res.results[c]["out_q"], dtype=np.float32)
        for j, g in enumerate(QSETS[s]):
            out[b, g * 128:(g + 1) * 128, :] = oq[j]
    return out


# revision 35
# speedup vs baseline: 1.0210x; 1.0210x over previous
"""Causal attention (B=4, N=2048, D=1024) on 8 Trainium2 NeuronCores.

v2 design (vs baseline):
  * All on-chip tensors bf16 (tolerance 2e-2; measured numpy pipeline err
    ~4e-3).  Halves DMA bytes and SBUF so K^T, V and Q^T stay fully
    SBUF-resident -- no DRAM spill roundtrips.
  * Scores computed TRANSPOSED (S^T[k,q] per key tile): the exp'd P^T is
    directly the stationary operand of the AV matmul, eliminating all PE
    transposes and the exp->transpose->copy->AV serial chain.  Row sums
    come from a 1-wide matmul against a ones vector that reuses the AV
    stationary (P^T) already loaded in the array.
  * Core 2b+s handles batch b; s=0 takes query tiles {0,2,4,6, 9,11,13,15},
    s=1 takes {1,3,5,7, 8,10,12,14} -- both sum to 68 causal key-tile pairs.
    The program is SPMD-uniform with key limits (2,4,..,16); the per-core
    diagonal/full masks are passed as input data ([128,512] = group1 pair +
    group2 pair of 128-col blocks).
  * Slot groups of 4 give 512-wide moving operands for S^T; widths taper
    (512/384/256/128) following the causal staircase.
  * x double-buffered across the two key-tile phases; weights loaded once.
"""
import sys

sys.path.insert(0, "/opt/trn_rl_repo")

from contextlib import ExitStack

import numpy as np
import ml_dtypes

import concourse.bass as bass
import concourse.mybir as mybir
import concourse.tile as tile
from concourse import bacc
from concourse.bass_utils import run_bass_kernel_spmd

B, N, D = 4, 2048, 1024
N_CORES = 8
N_SLOTS = 8
N_KTILES = 16
SCALE = 1.0 / 32.0   # 1/sqrt(D)
NEG = -1.0e9

F32 = mybir.dt.float32
BF16 = mybir.dt.bfloat16
BF = ml_dtypes.bfloat16

# query-tile sets per parity slot s (ascending); both have sum(g+1) == 68
QSETS = [
    [0, 2, 4, 6, 9, 11, 13, 15],
    [1, 3, 5, 7, 8, 10, 12, 14],
]
# uniform program limits per slot (key tiles 0..L-1 computed)
LIMITS = [2, 4, 6, 8, 10, 12, 14, 16]

_NC_CACHE = {}
TRACE = False
LAST_EXEC_NS = None


def _build_nc():
    nc = bacc.Bacc(None, target_bir_lowering=False, debug=False)

    # x tile layout: [tile, p=d%128, dchunk, token]
    x_t = nc.declare_dram_parameter("x_t", [N_KTILES, 128, 8, 128], BF16, isOutput=False)
    x_qt = nc.declare_dram_parameter("x_qt", [N_SLOTS, 128, 8, 128], BF16, isOutput=False)
    # weights: [p=d%128, dchunk, ecol]
    wq = nc.declare_dram_parameter("wq", [128, 8, 1024], BF16, isOutput=False)
    wk = nc.declare_dram_parameter("wk", [128, 8, 1024], BF16, isOutput=False)
    # wv is e-half-major so each half is one contiguous DMA on its own queue
    wv = nc.declare_dram_parameter("wv", [2, 128, 8, 512], BF16, isOutput=False)
    mask_in = nc.declare_dram_parameter("mask", [128, 512], F32, isOutput=False)
    out_q = nc.declare_dram_parameter("out_q", [N_SLOTS, 128, D], BF16, isOutput=True)

    with tile.TileContext(nc) as tc, ExitStack() as top:
        consts = top.enter_context(tc.tile_pool(name="consts", bufs=1))
        kt_pool = top.enter_context(tc.tile_pool(name="ktp", bufs=1))
        v_pool = top.enter_context(tc.tile_pool(name="vp", bufs=1))
        qt_pool = top.enter_context(tc.tile_pool(name="qtp", bufs=1))

        ones = consts.tile([128, 8], BF16)
        nc.vector.memset(ones, 1.0)
        mask_sb = consts.tile([128, 512], F32)


        KT = kt_pool.tile([128, 8, N], BF16)     # [p=e%128, echunk, key]
        V = v_pool.tile([128, N_KTILES, D], BF16)  # [p=key%128, ktile, e]
        QT = qt_pool.tile([128, 8, 1024], BF16)  # [p=e%128, echunk, qcol]

        with ExitStack() as ph12:
            xt_pool = ph12.enter_context(tc.tile_pool(name="xtp", bufs=2))
            qxt_pool = ph12.enter_context(tc.tile_pool(name="qxt", bufs=1))
            w_pool = ph12.enter_context(tc.tile_pool(name="wp", bufs=1))
            ps_mm = ph12.enter_context(tc.tile_pool(name="ps_mm", bufs=8, space="PSUM"))

            # spread weight DMAs across queues so they stream in parallel
            # (per-queue DMA BW is ~100-180 GB/s, well under core BW).
            # wv gates the kernel's first PE work: contiguous 1MB halves on
            # two queues land ~5us earlier than one 2MB transfer.
            # first wv half split across two queues (it gates the first PE
            # chain); second half + the rest stream behind
            wv_sb = w_pool.tile([128, 2, 8, 512], BF16, tag="wv")
            nc.scalar.dma_start(out=wv_sb[:, 0, 0:4, :], in_=wv[0][:, 0:4, :])
            nc.gpsimd.dma_start(out=wv_sb[:, 0, 4:8, :], in_=wv[0][:, 4:8, :])
            nc.sync.dma_start(out=wv_sb[:, 1], in_=wv[1][:, :, :])
            wk_sb = w_pool.tile([128, 8, 1024], BF16, tag="wk")
            wq_sb = w_pool.tile([128, 8, 1024], BF16, tag="wq")
            nc.sync.dma_start(out=wq_sb, in_=wq[:, :, :])

            QXT = qxt_pool.tile([128, 8, 8, 128], BF16, tag="qx")
            nc.sync.dma_start(
                out=QXT, in_=x_qt[:].rearrange("s p c q -> p s c q")
            )
            nc.sync.dma_start(out=mask_sb, in_=mask_in[:, :])

            def proj_phase(kh):
                xT = xt_pool.tile([128, 8, 8, 128], BF16, tag="xT", name=f"x{kh}")
                if kh == 0:
                    # per-tile DMAs: first V chain starts after one tile lands
                    for lt in range(8):
                        nc.gpsimd.dma_start(
                            out=xT[:, lt, :, :], in_=x_t[lt][:, :, :]
                        )
                    # wk rides the gpsimd queue behind the kh0 x tiles;
                    # K proj needs it only after the 8 V chains (~25us in)
                    nc.gpsimd.dma_start(out=wk_sb, in_=wk[:, :, :])
                else:
                    # bulk prefetch on the scalar queue (free after wv)
                    nc.scalar.dma_start(
                        out=xT, in_=x_t[8:16].rearrange("t p c q -> p t c q")
                    )
                # V projection: stationary x chunk shared across both e-halves.
                # The first two kh0 tiles run e-halves sequentially so the
                # first chain depends only on the first wv half-DMA.
                for lt in range(8):
                    t = kh * 8 + lt
                    vps = [ps_mm.tile([128, 512], F32, tag="mm", name=f"v{t}_{eh}")
                           for eh in range(2)]
                    if kh == 0 and lt < 2:
                        for eh in range(2):
                            for c in range(8):
                                nc.tensor.matmul(
                                    vps[eh], xT[:, lt, c, :], wv_sb[:, eh, c, :],
                                    start=(c == 0), stop=(c == 7),
                                )
                    else:
                        for c in range(8):
                            for eh in range(2):
                                nc.tensor.matmul(
                                    vps[eh], xT[:, lt, c, :], wv_sb[:, eh, c, :],
                                    start=(c == 0), stop=(c == 7),
                                )
                    for eh in range(2):
                        nc.vector.tensor_copy(V[:, t, eh * 512:(eh + 1) * 512], vps[eh])
                # K^T projection: stationary W chunk shared across both key groups
                for e in range(8):
                    kps = [ps_mm.tile([128, 512], F32, tag="mm", name=f"k{kh}_{e}_{g}")
                           for g in range(2)]
                    for c in range(8):
                        for kg in range(2):
                            nc.tensor.matmul(
                                kps[kg], wk_sb[:, c, e * 128:(e + 1) * 128],
                                xT[:, kg * 4:(kg + 1) * 4, c, :],
                                start=(c == 0), stop=(c == 7),
                            )
                    for kg in range(2):
                        key0 = (kh * 8 + kg * 4) * 128
                        nc.vector.tensor_copy(KT[:, e, key0:key0 + 512], kps[kg])

            def proj_queries():
                # both slot groups; stationary W chunk shared across groups
                for e in range(8):
                    qps = [ps_mm.tile([128, 512], F32, tag="mm", name=f"q{e}_{g}")
                           for g in range(2)]
                    for c in range(8):
                        for g in range(2):
                            nc.tensor.matmul(
                                qps[g], wq_sb[:, c, e * 128:(e + 1) * 128],
                                QXT[:, g * 4:(g + 1) * 4, c, :],
                                start=(c == 0), stop=(c == 7),
                            )
                    for g in range(2):
                        nc.vector.tensor_copy(QT[:, e, g * 512:(g + 1) * 512], qps[g])

            proj_phase(0)
            proj_queries()
            proj_phase(1)

        # ---- attention: S^T per key tile, then AV with P^T stationary ----
        with ExitStack() as ph3:
            pt_pool = ph3.enter_context(tc.tile_pool(name="ptp", bufs=1))
            ps_st = ph3.enter_context(tc.tile_pool(name="ps_st", bufs=3, space="PSUM"))
            ps_o = ph3.enter_context(tc.tile_pool(name="ps_o", bufs=2, space="PSUM"))
            ps_rs = ph3.enter_context(tc.tile_pool(name="ps_rs", bufs=1, space="PSUM"))
            sc_pool = ph3.enter_context(tc.tile_pool(name="scp", bufs=2))
            outp = ph3.enter_context(tc.tile_pool(name="outp", bufs=2))

            PTs = [
                pt_pool.tile([128, 8, 512], BF16, tag="pt1", name="PT1"),
                pt_pool.tile([128, 16, 512], BF16, tag="pt2", name="PT2"),
            ]

            def st_fused():
                # one pass over key tiles; each KT stationary chunk serves
                # BOTH slot groups' S^T matmuls (halves LDWEIGHTS for kt<8)
                for kt in range(16):
                    work = []   # (group, sps, w, col0, f)
                    for g in ((1, 0) if kt < 8 else (1,)):
                        Ls = LIMITS[g * 4:(g + 1) * 4]
                        f = sum(1 for L in Ls if L <= kt)
                        w = (4 - f) * 128
                        col0 = f * 128
                        sps = ps_st.tile([128, 512], F32, tag="st",
                                         name=f"s{g}_{kt}")
                        work.append((g, sps, w, col0, f))
                    for c in range(8):
                        for g, sps, w, col0, f in work:
                            nc.tensor.matmul(
                                sps[:, 0:w],
                                KT[:, c, kt * 128:(kt + 1) * 128],
                                QT[:, c, g * 512 + col0: g * 512 + col0 + w],
                                start=(c == 0), stop=(c == 7),
                            )
                    for g, sps, w, col0, f in work:
                        Ls = LIMITS[g * 4:(g + 1) * 4]
                        if kt == Ls[f] - 2:
                            nc.vector.tensor_add(
                                sps[:, 0:128], sps[:, 0:128],
                                mask_sb[:, g * 256: g * 256 + 128],
                            )
                        elif kt == Ls[f] - 1:
                            nc.vector.tensor_add(
                                sps[:, 0:128], sps[:, 0:128],
                                mask_sb[:, g * 256 + 128: g * 256 + 256],
                            )
                        nc.scalar.activation(
                            PTs[g][:, kt, col0:col0 + w], sps[:, 0:w],
                            mybir.ActivationFunctionType.Exp,
                            bias=0.0, scale=SCALE,
                        )

            def av_slot(g, j):
                PT = PTs[g]
                slot = g * 4 + j
                L = LIMITS[slot]
                col = j * 128
                O_ps = ps_o.tile([128, D], F32, tag="O", name=f"O{slot}")
                rs_ps = ps_rs.tile([128, 1], F32, tag="rs", name=f"r{slot}")
                for kt in range(L):
                    pt_blk = PT[:, kt, col:col + 128]
                    for h in range(2):
                        nc.tensor.matmul(
                            O_ps[:, h * 512:(h + 1) * 512], pt_blk,
                            V[:, kt, h * 512:(h + 1) * 512],
                            start=(kt == 0), stop=(kt == L - 1),
                        )
                    nc.tensor.matmul(
                        rs_ps, pt_blk, ones[:, 0:1],
                        start=(kt == 0), stop=(kt == L - 1),
                    )
                stats = sc_pool.tile([128, 8], F32, tag="stats", name=f"st{slot}")
                recip = stats[:, 0:1]
                nc.vector.reciprocal(recip, rs_ps)
                out_sb = outp.tile([128, D], BF16, tag="osb", name=f"ou{slot}")
                nc.vector.tensor_scalar_mul(out_sb, O_ps, recip)
                # alternate output queues so the final drain is parallel
                eng = nc.sync if slot % 2 == 0 else nc.gpsimd
                eng.dma_start(out=out_q[slot][:, :], in_=out_sb)

            # descending L within each group: the big slots' outputs DMA out
            # early, shrinking the end-of-kernel drain
            # interleave big(g2)/small(g1) slots in descending L: each small
            # slot's recip/scale/DMA epilogue hides under the next big slot's
            # matmul chain, and the last emitted slot (L=2) has the shortest
            # end-of-kernel chain
            st_fused()
            for j in (3, 2, 1, 0):
                av_slot(1, j)
                av_slot(0, j)

    nc.compile()
    return nc


def _masks():
    k = np.arange(128)[:, None]
    q = np.arange(128)[None, :]
    tril_t = np.where(k <= q, 0.0, NEG).astype(np.float32)  # S^T diag block
    fullneg = np.full((128, 128), NEG, np.float32)
    zeros = np.zeros((128, 128), np.float32)
    m_s0 = np.concatenate([tril_t, fullneg, zeros, tril_t], axis=1)
    m_s1 = np.concatenate([zeros, tril_t, tril_t, fullneg], axis=1)
    return m_s0, m_s1


def kernel(x, Wq, Wk, Wv):
    global LAST_EXEC_NS
    x = np.asarray(x, dtype=np.float32)
    Wq = np.asarray(Wq, dtype=np.float32)
    Wk = np.asarray(Wk, dtype=np.float32)
    Wv = np.asarray(Wv, dtype=np.float32)

    if "nc" not in _NC_CACHE:
        _NC_CACHE["nc"] = _build_nc()
    nc = _NC_CACHE["nc"]

    # host pre-transpose: x[b] (N, D) -> (tile, p=d%128, dchunk, token), bf16
    xt_all = np.ascontiguousarray(
        x.reshape(B, N_KTILES, 128, 8, 128).transpose(0, 1, 4, 3, 2).astype(BF)
    )  # [B, tile, p, c, q]

    # weights -> [p=d%128, dchunk, ecol], bf16
    wq_r = np.ascontiguousarray(Wq.reshape(8, 128, 1024).transpose(1, 0, 2).astype(BF))
    wk_r = np.ascontiguousarray(Wk.reshape(8, 128, 1024).transpose(1, 0, 2).astype(BF))
    # wv: [eh, p=d%128, dchunk, ecol]
    wv_r = np.ascontiguousarray(
        Wv.reshape(8, 128, 2, 512).transpose(2, 1, 0, 3).astype(BF))

    m_s0, m_s1 = _masks()
    in_maps = []
    for c in range(N_CORES):
        b, s = divmod(c, 2)
        in_maps.append({
            "x_t": xt_all[b],
            "x_qt": np.ascontiguousarray(xt_all[b, QSETS[s]]),
            "wq": wq_r, "wk": wk_r, "wv": wv_r,
            "mask": m_s1 if s else m_s0,
        })

    res = run_bass_kernel_spmd(nc, in_maps, list(range(N_CORES)), trace=TRACE)
    LAST_EXEC_NS = res.exec_time_ns

    out = np.empty((B, N, D), dtype=np.float32)
    for c in range(N_CORES):
        b, s = divmod(c, 2)
        oq = np.asarray(res.results[c]["out_q"], dtype=np.float32)
        for j, g in enumerate(QSETS[s]):
            out[b, g * 128:(g + 1) * 128, :] = oq[j]
    return out
